# revision 26
# baseline (speedup 1.0000x reference)
"""Trainium2 Bass kernel for DinoVisionTransformer Sparse-MoE FC2 (LoRA experts).

Computation (per token t):
    logits = x @ Wg                      -> top-2 softmax-renormalized weights
    out    = x @ W2 + b2 + sum_e cw[t,e] * scale[e] * (x @ A_e) @ B_e

Sharding: data-parallel over the batch dim (8 batch rows -> 8 NeuronCores,
1024 tokens each). All weights replicated.

Per-core kernel (fp16 compute, fp32 PSUM accumulation):
  Phase A (per 128-token tile, contraction over H=4096 in 32 k-chunks of 128):
    stationary = xT tile [128h x 128t]; moving = Wcat[k] columns where
    Wcat = [W2 (1024) | A_flat (512) | Wg_hi (8) | Wg_lo (8)]  (fp16)
    -> psum_base [128,1024], psum_h [128,512], psum_L [128,16]
    Router logits get near-fp32 precision via the split x = x_hi + x_lo:
    logits = x_hi@Wg_hi + x_hi@Wg_lo + x_lo@Wg_hi  (all accumulated in fp32)
  Router (DVE): top-2 of 8 via max8; w1 = sigmoid(l1-l2), w2 = 1-w1;
    dense combine weights cw[t,e] by equality masks. scale[] folded into Bm.
  LoRA: h weighted by cw, transposed 128x128 via PE, then phase B matmuls
    (contraction over E*R=512) accumulate directly into psum_base.
  Final: out = psum_base + b2 (broadcast), DMA to DRAM.
"""

import sys

if "/opt/trn_rl_repo" not in sys.path:
    sys.path.insert(0, "/opt/trn_rl_repo")

import numpy as np

import concourse.bass as bass  # noqa: F401  (registers types)
import concourse.mybir as mybir
import concourse.tile as tile
from concourse import bacc
from concourse.bass import ts
from concourse.bass_utils import run_bass_kernel_spmd
from concourse.masks import make_identity
from concourse.tile import add_dep_helper

P = 128
KCH = 32          # H / 128 contraction chunks
TT = 8            # 128-token tiles per core
H = 4096
D = 1024
E = 8
R = 64
ER = E * R        # 512
NW = D + ER + 8 + 8   # 1552 wcat columns
NCORES = 8
WG_K_GROUPS = 16  # wcat DMA split granularity (k-chunks per group)
KPG = KCH // WG_K_GROUPS  # 2

F16 = mybir.dt.float16
F32 = mybir.dt.float32
F8 = mybir.dt.float8e4
XLO_SCALE = 1024.0

_CACHE = {}


def _build_nc():
    nc = bacc.Bacc("TRN2")

    xtb_d = nc.dram_tensor("xtb", [TT, P, KCH, P], F16, kind="ExternalInput")
    xlo_d = nc.dram_tensor("xlo", [TT, P, KCH, P], F16, kind="ExternalInput")
    wcat_d = nc.dram_tensor("wcat", [P, KCH, NW], F16, kind="ExternalInput")
    bm_d = nc.dram_tensor("bm", [P, 4, D], F16, kind="ExternalInput")
    b2b_d = nc.dram_tensor("b2b", [P, D], F32, kind="ExternalInput")
    y_d = nc.dram_tensor("y", [TT * P, D], F32, kind="ExternalOutput")

    Sig = mybir.ActivationFunctionType.Sigmoid
    Alu = mybir.AluOpType

    with tile.TileContext(nc) as tc:
        with (
            tc.tile_pool(name="wres", bufs=1) as wres,
            tc.tile_pool(name="xin", bufs=3) as xin,
            tc.tile_pool(name="small", bufs=2) as small,
            tc.tile_pool(name="hbuf", bufs=2) as hbuf,
            tc.tile_pool(name="obuf", bufs=2) as obuf,
            tc.tile_pool(name="ps_base", bufs=2, space="PSUM") as ps_base_pool,
            tc.tile_pool(name="ps_h", bufs=2, space="PSUM") as ps_h_pool,
            tc.tile_pool(name="ps_l", bufs=1, space="PSUM") as ps_l_pool,
            tc.tile_pool(name="ps_t", bufs=1, space="PSUM") as ps_t_pool,
        ):
            # ---- startup loads. HWDGE drains roughly in issue order, so
            # interleave x_hi chunks of tiles 0/1 with the first wcat chunks
            # to match the PE's consumption order; everything else after. ----
            xts = {}
            xlo01 = {}
            xtb01 = {0: [], 1: []}
            for t0 in (0, 1):
                for c in range(4):
                    t_ = wres.tile([P, 8, P], F16, tag=f"xtb{t0}c{c}")
                    xtb01[t0].append(t_)
                xlo_ = xin.tile([P, KCH, P], F16, tag="xlo")
                xlo01[t0] = xlo_
                xts[t0] = (xtb01[t0], xlo_)
            wcat_sb = []
            for c in range(4):
                nc.sync.dma_start(xtb01[0][c][:], xtb_d[0, :, ts(c, 8), :])
                nc.sync.dma_start(xtb01[1][c][:], xtb_d[1, :, ts(c, 8), :])
                t_ = wres.tile([P, KPG, NW], F16, tag=f"wcat{c}")
                nc.sync.dma_start(t_[:], wcat_d[:, ts(c, KPG), :])
                wcat_sb.append(t_)
            nc.sync.dma_start(xlo01[0][:], xlo_d[0])
            nc.sync.dma_start(xlo01[1][:], xlo_d[1])
            bm_sb = wres.tile([P, 4, D], F16, tag="bm")
            nc.sync.dma_start(bm_sb[:], bm_d[:])
            b2b_sb = wres.tile([P, D], F32, tag="b2b")
            nc.sync.dma_start(b2b_sb[:], b2b_d[:])
            for g in range(4, WG_K_GROUPS):
                t_ = wres.tile([P, KPG, NW], F16, tag=f"wcat{g}")
                nc.sync.dma_start(t_[:], wcat_d[:, ts(g, KPG), :])
                wcat_sb.append(t_)
            for t0 in (2, 3):
                xtb_ = xin.tile([P, KCH, P], F16, tag="xtb")
                xlo_ = xin.tile([P, KCH, P], F16, tag="xlo")
                nc.sync.dma_start(xtb_[:], xtb_d[t0])
                nc.sync.dma_start(xlo_[:], xlo_d[t0])
                xts[t0] = (xtb_, xlo_)
            ident = wres.tile([P, P], F16, tag="ident")
            make_identity(nc, ident[:])

            def wc(k, lo, hi):
                return wcat_sb[k // KPG][:, k % KPG, lo:hi]

            # shared logits psum bank: tile t uses half (t % 2)
            ps_l_shared = ps_l_pool.tile([P, 32], F32, tag="l")

            pend = {}   # t -> (ps_base, ps_h, hwT or None)

            def emit_A_group(t, g, late_xlo=False, warm_only=False):
                """Phase-A matmuls for k-chunks [g*KPG, (g+1)*KPG) of tile t.

                late_xlo: bunch the xlo-correction matmuls into the second
                half of the k-loop (two per slot) so the xlo DMA can be
                issued after the first wcat chunks during startup."""
                xtb_sb, xlo_sb = xts[t]
                if isinstance(xtb_sb, list):
                    def xap(k, _x=xtb_sb):
                        return _x[k // 8][:, k % 8, :]
                else:
                    def xap(k, _x=xtb_sb):
                        return _x[:, k, :]
                ps_base, ps_h, _ = pend[t]
                ps_l = ps_l_shared[:, (t % 2) * 16:(t % 2) * 16 + 16]
                for k in range(g * KPG, (g + 1) * KPG):
                    st = k == 0
                    fin = k == KCH - 1
                    # order: tiny-N matmuls sit between 512-col streams so
                    # their self-loading weight fetches hide under the streams
                    nc.tensor.matmul(
                        ps_base[:, 0:512], xap(k), wc(k, 0, 512),
                        start=st, stop=False, skip_group_check=True,
                    )
                    if not warm_only:
                        nc.tensor.matmul(
                            ps_l[:, 0:16], xap(k), wc(k, 1536, 1552),
                            start=False, stop=False, skip_group_check=True,
                        )
                    nc.tensor.matmul(
                        ps_base[:, 512:1024], xap(k), wc(k, 512, 1024),
                        start=st, stop=False, skip_group_check=True,
                    )
                    nc.tensor.matmul(
                        ps_h[:, :], xap(k), wc(k, 1024, 1536),
                        start=st, stop=fin, skip_group_check=True,
                    )
                    if warm_only:
                        continue
                    if late_xlo:
                        if k >= KCH // 2:
                            for kx in (k - KCH // 2, k):
                                nc.tensor.matmul(
                                    ps_l[:, 0:8], xlo_sb[:, kx, :],
                                    wc(kx, 1536, 1544),
                                    start=False, stop=(kx == KCH - 1),
                                    skip_group_check=True,
                                )
                    else:
                        nc.tensor.matmul(
                            ps_l[:, 0:8], xlo_sb[:, k, :], wc(k, 1536, 1544),
                            start=False, stop=fin, skip_group_check=True,
                        )

            def emit_router(t):
                """Router + h-weighting + transpose; fills pend[t] hwT."""
                ps_base, ps_h, _ = pend[t]
                ps_l = ps_l_shared[:, (t % 2) * 16:(t % 2) * 16 + 16]
                logits = small.tile([P, 8], F32, tag="logits")
                nc.vector.tensor_reduce(
                    logits[:],
                    ps_l.rearrange("p (s j) -> p j s", s=2),
                    axis=mybir.AxisListType.X,
                    op=Alu.add,
                )
                m8 = small.tile([P, 8], F32, tag="m8")
                nc.vector.max(m8[:], logits[:])
                g_ = small.tile([P, 1], F32, tag="gap")
                nc.vector.tensor_sub(g_[:], m8[:, 0:1], m8[:, 1:2])
                w1 = small.tile([P, 1], F32, tag="w1")
                nc.scalar.activation(w1[:], g_[:], Sig)
                w2 = small.tile([P, 1], F32, tag="w2")
                nc.scalar.activation(w2[:], g_[:], Sig, scale=-1.0)
                cw = small.tile([P, 8], F32, tag="cw")
                cwb = small.tile([P, 8], F32, tag="cwb")
                nc.vector.scalar_tensor_tensor(
                    cw[:], logits[:], m8[:, 0:1], w1[:, 0:1].to_broadcast([P, 8]),
                    op0=Alu.is_equal, op1=Alu.mult,
                )
                nc.vector.scalar_tensor_tensor(
                    cwb[:], logits[:], m8[:, 1:2], w2[:, 0:1].to_broadcast([P, 8]),
                    op0=Alu.is_equal, op1=Alu.mult,
                )
                nc.vector.tensor_add(cw[:], cw[:], cwb[:])
                hw = hbuf.tile([P, ER], F16, tag="hw")
                nc.vector.tensor_tensor(
                    hw.rearrange("p (e r) -> p e r", e=E),
                    ps_h.rearrange("p (e r) -> p e r", e=E),
                    cw[:, :, None].to_broadcast([P, E, R]),
                    Alu.mult,
                )
                ps_t = ps_t_pool.tile([P, ER], F16, tag="t")
                for j in range(4):
                    nc.tensor.transpose(
                        ps_t[:, ts(j, P)], hw[:, ts(j, P)], ident[:]
                    )
                hwT = hbuf.tile([P, 4, P], F16, tag="hwT")
                nc.vector.tensor_copy(hwT.rearrange("p a b -> p (a b)"), ps_t[:])
                pend[t] = (ps_base, ps_h, hwT)

            def emit_B_and_out(t):
                """LoRA phase B accumulated into base psum, bias add, store."""
                ps_base, _, hwT = pend.pop(t)
                for j in range(4):
                    nc.tensor.matmul(
                        ps_base[:, 0:512], hwT[:, j, :], bm_sb[:, j, 0:512],
                        start=False, stop=False, skip_group_check=True,
                    )
                    nc.tensor.matmul(
                        ps_base[:, 512:1024], hwT[:, j, :], bm_sb[:, j, 512:1024],
                        start=False, stop=(j == 3), skip_group_check=True,
                    )
                out_sb = obuf.tile([P, D], F32, tag="out")
                nc.vector.tensor_add(out_sb[:], ps_base[:], b2b_sb[:])
                nc.scalar.dma_start(y_d[ts(t, P), :], out_sb[:])

            def alloc_psums(t):
                pend[t] = (
                    ps_base_pool.tile([P, D], F32, tag="base", name=f"base{t}"),
                    ps_h_pool.tile([P, ER], F32, tag="h", name=f"h{t}"),
                    None,
                )
                # The shared logits bank must never see start=True (a bank-wide
                # has_written clear would wipe the other tile's half). Instead
                # zero this tile's half; start=False matmuls then accumulate
                # onto 0 (bits set) or overwrite with v (bits clear) — both ok.
                nc.vector.memset(
                    ps_l_shared[:, (t % 2) * 16:(t % 2) * 16 + 16], 0.0
                )

            # ---- startup: interleave phase A of tiles 0 and 1 so the PE has
            # two tiles of work while wcat chunks stream in ----
            D_OFF = 2
            alloc_psums(0)
            alloc_psums(1)
            # warm-up replays: re-run group 0 (start=True re-clears, so the
            # final real pass wins). Fills DMA-wait gaps and warms the PE
            # clock (HAM) before the real work lands.
            for _ in range(2):
                emit_A_group(0, 0, warm_only=True)
            for g in range(WG_K_GROUPS + D_OFF):
                if g == D_OFF:
                    for _ in range(2):
                        emit_A_group(1, 0, warm_only=True)
                if g < WG_K_GROUPS:
                    emit_A_group(0, g, late_xlo=True)
                if g == WG_K_GROUPS:
                    emit_router(0)
                gg = g - D_OFF
                if 0 <= gg < WG_K_GROUPS:
                    emit_A_group(1, gg, late_xlo=True)
                if gg == WG_K_GROUPS - 2:
                    emit_B_and_out(0)
            emit_router(1)

            # ---- steady state ----
            for t in range(2, TT):
                if t >= 4:
                    xtb_ = xin.tile([P, KCH, P], F16, tag="xtb")
                    xlo_ = xin.tile([P, KCH, P], F16, tag="xlo")
                    nc.sync.dma_start(xtb_[:], xtb_d[t])
                    nc.sync.dma_start(xlo_[:], xlo_d[t])
                    xts[t] = (xtb_, xlo_)
                alloc_psums(t)
                for g in range(WG_K_GROUPS):
                    emit_A_group(t, g)
                    if g == 4:
                        # previous tile's phase B mid-A so its psum/base slot
                        # frees well before tile t+1 needs it
                        emit_B_and_out(t - 1)
                emit_router(t)
            # drain: last tile's phase B with split evac so the first half's
            # bias-add + store overlap the second half's matmuls
            ps_base, _, hwT = pend.pop(TT - 1)
            for j in range(4):
                nc.tensor.matmul(
                    ps_base[:, 0:512], hwT[:, j, :], bm_sb[:, j, 0:512],
                    start=False, stop=(j == 3), skip_group_check=True,
                )
            out_sb = obuf.tile([P, D], F32, tag="out")
            nc.vector.tensor_add(
                out_sb[:, 0:512], ps_base[:, 0:512], b2b_sb[:, 0:512]
            )
            nc.scalar.dma_start(
                y_d[ts(TT - 1, P), 0:512], out_sb[:, 0:512]
            )
            for j in range(4):
                nc.tensor.matmul(
                    ps_base[:, 512:1024], hwT[:, j, :], bm_sb[:, j, 512:1024],
                    start=False, stop=(j == 3), skip_group_check=True,
                )
            nc.vector.tensor_add(
                out_sb[:, 512:1024], ps_base[:, 512:1024], b2b_sb[:, 512:1024]
            )
            nc.scalar.dma_start(
                y_d[ts(TT - 1, P), 512:1024], out_sb[:, 512:1024]
            )

    nc.finalize()
    return nc


def _prep_shared(Wg, W2, b2, A, Bm, scale):
    """Host-side weight layout prep (replicated across cores)."""
    f16, f32 = np.float16, np.float32
    # Wcat = [W2 | A_flat | Wg_hi | Wg_lo], k-chunked to [128, 32, NW]
    a_flat = np.ascontiguousarray(A.transpose(1, 0, 2)).reshape(H, ER)
    wg_hi = Wg.astype(f16)
    wg_lo = (Wg.astype(f32) - wg_hi.astype(f32)).astype(f16)
    wcat = np.empty((H, NW), dtype=f16)
    wcat[:, 0:D] = W2.astype(f16)
    wcat[:, D:D + ER] = a_flat.astype(f16)
    wcat[:, D + ER:D + ER + 8] = wg_hi
    wcat[:, D + ER + 8:] = wg_lo
    wcat = np.ascontiguousarray(wcat.reshape(KCH, P, NW).transpose(1, 0, 2))

    # Bm with scale folded, [(e r), d] -> [128, 4, D]
    bms = (Bm.astype(f32) * scale.astype(f32)[:, None, None]).reshape(ER, D)
    bms = np.ascontiguousarray(bms.reshape(4, P, D).transpose(1, 0, 2)).astype(f16)

    b2b = np.ascontiguousarray(
        np.broadcast_to(b2.astype(f32)[None, :], (P, D))
    )
    return wcat, bms, b2b


def _prep_x_core(x_c):
    """Per-core x prep: fp16 hi + scaled-fp8 lo split, [tile, p, k, ti] layout."""
    f16, f32 = np.float16, np.float32
    xtb = x_c.astype(f16)                                   # [1024, 4096]
    xlo = (x_c.astype(f32) - xtb.astype(f32)).astype(f16)
    def lay(a):
        return np.ascontiguousarray(
            a.reshape(TT, P, KCH, P).transpose(0, 3, 2, 1)
        )
    return lay(xtb), lay(xlo)


def kernel(x, Wg, W2, b2, A, Bm, scale):
    x = np.asarray(x, dtype=np.float32)
    Wg = np.asarray(Wg, dtype=np.float32)
    W2 = np.asarray(W2, dtype=np.float32)
    b2 = np.asarray(b2, dtype=np.float32)
    A = np.asarray(A, dtype=np.float32)
    Bm = np.asarray(Bm, dtype=np.float32)
    scale = np.asarray(scale, dtype=np.float32)

    if "nc" not in _CACHE:
        _CACHE["nc"] = _build_nc()
    nc = _CACHE["nc"]

    wcat, bms, b2b = _prep_shared(Wg, W2, b2, A, Bm, scale)
    in_maps = []
    for c in range(NCORES):
        xtb, xlo = _prep_x_core(x[c])
        in_maps.append(
            {"xtb": xtb, "xlo": xlo, "wcat": wcat, "bm": bms, "b2b": b2b}
        )

    res = run_bass_kernel_spmd(nc, in_maps, core_ids=list(range(NCORES)))
    out = np.stack([res.results[c]["y"] for c in range(NCORES)], axis=0)
    return out.astype(np.float32)
